# revision 1
# baseline (speedup 1.0000x reference)
"""Trainium2 Bass kernel for nn_Attn (Bahdanau-style attention scores).

Computation (per batch b of B=128):
    energy = tanh(enc[b] @ We.T + (hidden @ Wh.T)[b] + bias)   # (L, H)
    scores = energy @ v                                        # (L,)
    out[b] = softmax(scores)                                   # (1, L)

Sharding: batch data-parallel over 8 NeuronCores (16 batches/core);
weights replicated. Per core the dominant matmul is computed in the
[h, l] orientation so the PE tensor engine contracts over d (=576):

    part_e[h, l] = sum_d WeT[d, h] * encT[d, l]      (lhsT=WeT, rhs=encT)

which lets the (hidden@Wh.T + bias) term fuse into the tanh as a
per-partition activation bias, and the v-contraction run as a second
PE matmul (lhsT = v column, contracting over h on partitions).
Matmuls run as float32r (full fp32 data, reduced-precision multiply,
1 col/cycle on TRN2 vs 4 for exact fp32).

The contraction dim is zero-padded host-side from 576 to 640 so every
k-tile is a full 128 partitions: K=64 matmuls (and their successors)
measure ~2x slower on HW than K=128 ones, costing far more than the 11%
extra DMA.

Host side: encoder_outputs (L, B, D) is transposed once to (B, D, L) so
each per-batch d-major tile DMA is contiguous.

Scores are assembled batch-major ([16, L] via tiny SBUF->SBUF row DMAs)
so softmax runs once over all local batches at the end instead of as 16
serial per-batch chains on the ACT/DVE engines.
"""

import numpy as np

import concourse.bacc as bacc
import concourse.bass as bass
import concourse.mybir as mybir
import concourse.tile as tile
from concourse import bass_utils
from concourse.mybir import ActivationFunctionType as AF
from concourse.mybir import AluOpType, AxisListType

N_CORES = 8
B, L, H = 128, 1024, 512
ONEHOT = 64
DE = H + ONEHOT          # 576, true contraction dim of the big matmul
DP = 640                 # padded contraction dim (5 full 128-tiles)
BL = B // N_CORES        # 16 batches per core
F32 = mybir.dt.float32
F32R = mybir.dt.float32r

NKT = DP // 128                          # 5 d-tiles, all full
NHT = H // 128                           # 4 h-tiles
NLH = L // 512                           # 2 l-halves (N=512 per matmul)


BF16 = mybir.dt.bfloat16


def build(reps: int = 1, dt1=F32R, dt2=F32R, dve2: bool = True):
    """Build + trace the per-core Bass program. Returns the compiled nc.

    dt1: dtype of the stage-1 matmul operands (enc tiles + We tiles).
    dt2: dtype of the stage-2 operands (energy tiles + v columns).
    dve2: compute z[p,l] = sum_ht v_ht[p]*en_ht[p,l] on the VectorE
        (per-partition scalar multiply-accumulate), so stage-2 on the PE
        collapses from 4 matmuls to a single K=128 ones-matmul per
        (batch, l-half). Saves ~23us of PE time for ~45us of idle DVE.
    """
    nc = bacc.Bacc(
        "TRN2", target_bir_lowering=False, debug=False, num_devices=N_CORES
    )
    enc = nc.dram_tensor("enc", [BL, DP, L], dt1, kind="ExternalInput").ap()
    hid = nc.dram_tensor("hid", [H, BL], F32, kind="ExternalInput").ap()
    wet = nc.dram_tensor("wet", [DP, H], dt1, kind="ExternalInput").ap()
    wht = nc.dram_tensor("wht", [H, H], F32, kind="ExternalInput").ap()
    bcol = nc.dram_tensor("bcol", [128, NHT], F32, kind="ExternalInput").ap()
    vcol = nc.dram_tensor("vcol", [128, NHT], F32 if dve2 else dt2, kind="ExternalInput").ap()
    ones = None
    vcolr = None
    if dve2:
        ones = nc.dram_tensor("ones", [128, 1], dt2, kind="ExternalInput").ap()
        vcolr = nc.dram_tensor("vcolr", [128, NHT], dt2, kind="ExternalInput").ap()
    out = nc.dram_tensor("out", [BL, L], F32, kind="ExternalOutput").ap()

    with tile.TileContext(nc) as tc:
        with (
            tc.tile_pool(name="const", bufs=1) as cpool,
            tc.tile_pool(name="encp", bufs=4) as epool,
            tc.tile_pool(name="energy", bufs=8) as gpool,
            tc.tile_pool(name="cb", bufs=2) as cbpool,
            tc.tile_pool(name="soft", bufs=1) as spool,
            tc.tile_pool(name="stage", bufs=4) as stpool,
            tc.tile_pool(name="ps1", bufs=6, space="PSUM") as ps1,
            tc.tile_pool(name="ps2", bufs=2, space="PSUM") as ps2,
            tc.tile_pool(name="ps3", bufs=2, space="PSUM") as ps3,
        ):
            # ---- replicated constants (gpsimd queue: don't serialize
            # behind the big enc prefetches on the sync queue) ----
            wet_sb = []
            for kt in range(NKT):
                t = cpool.tile([128, H], dt1, tag=f"wet{kt}", name=f"wet{kt}")
                nc.sync.dma_start(t[:], wet[kt * 128 : (kt + 1) * 128, :])
                wet_sb.append(t)
            wht_sb = []
            for kt in range(4):
                t = cpool.tile([128, H], F32, tag=f"wht{kt}", name=f"wht{kt}")
                nc.sync.dma_start(t[:], wht[kt * 128 : (kt + 1) * 128, :])
                wht_sb.append(t)
            hid_sb = []
            for kt in range(4):
                t = cpool.tile([128, BL], F32, tag=f"hid{kt}", name=f"hid{kt}")
                nc.sync.dma_start(t[:], hid[kt * 128 : (kt + 1) * 128, :])
                hid_sb.append(t)
            bcol_sb = cpool.tile([128, NHT], F32, tag="bcol", name="bcol_sb")
            nc.sync.dma_start(bcol_sb[:], bcol[:, :])
            vcol_sb = cpool.tile([128, NHT], F32 if dve2 else dt2, tag="vcol", name="vcol_sb")
            nc.sync.dma_start(vcol_sb[:], vcol[:, :])
            ones128 = None
            vcolr_sb = None
            if dve2:
                ones128 = cpool.tile([128, 1], dt2, tag="ones128", name="ones128")
                nc.sync.dma_start(ones128[:], ones[:, :])
                vcolr_sb = cpool.tile([128, NHT], dt2, tag="vcolr", name="vcolr_sb")
                nc.sync.dma_start(vcolr_sb[:], vcolr[:, :])

            for _rep in range(reps):
                # ---- PE warmup: high-duty junk matmuls as soon as wet lands,
                # so the HAM clock-gate reaches 8/8 before real work. The
                # N=16 c matmuls below have ~3% array duty and never warm it.
                # 36 matmuls bridge the PE-idle window until batch 0's enc
                # tiles land (~15us): a shorter burst lets the HAM re-throttle
                # during the wait and the first ~25us of real matmuls run at
                # 1.2GHz.
                warm = ps1.tile([128, 512], F32, tag="ps1", name="warm")
                for w in range(36):
                    nc.tensor.matmul(
                        warm[:],
                        lhsT=wet_sb[0][:, 0:128],
                        rhs=wet_sb[0][:],
                        start=(w == 0),
                        stop=(w == 35),
                    )

                # ---- c[h, b] = (hidden @ Wh.T).T + bias, per-partition h ----
                cb_sb = []
                for ht in range(4):
                    pc = ps1.tile([128, 512], F32, tag="ps1", name=f"pc{ht}")
                    for kt in range(4):
                        nc.tensor.matmul(
                            pc[:, :BL],
                            lhsT=wht_sb[kt][:, ht * 128 : (ht + 1) * 128],
                            rhs=hid_sb[kt][:],
                            start=(kt == 0),
                            stop=(kt == 3),
                        )
                    cbt = cbpool.tile([128, BL], F32, tag=f"cb{ht}", name=f"cb{ht}")
                    nc.vector.tensor_scalar_add(
                        cbt[:], pc[:, :BL], bcol_sb[:, ht : ht + 1]
                    )
                    cb_sb.append(cbt)

                scores_sb = spool.tile([BL, L], F32, tag="scores", name="scores_sb")

                # ---- main loop over local batches ----
                for b in range(BL):
                    et = []
                    for kt in range(NKT):
                        t = epool.tile([128, L], dt1, tag=f"enc{kt}", name=f"enc{kt}_{b}")
                        nc.sync.dma_start(
                            t[:], enc[b, kt * 128 : (kt + 1) * 128, :]
                        )
                        et.append(t)

                    for lh in range(NLH):
                        ens = []
                        for ht in range(4):
                            pe_t = ps1.tile(
                                [128, 512], F32, tag="ps1", name=f"pe{b}_{lh}_{ht}"
                            )
                            for kt in range(NKT):
                                nc.tensor.matmul(
                                    pe_t[:],
                                    lhsT=wet_sb[kt][:, ht * 128 : (ht + 1) * 128],
                                    rhs=et[kt][:, lh * 512 : (lh + 1) * 512],
                                    start=(kt == 0),
                                    stop=(kt == NKT - 1),
                                )
                            en_t = gpool.tile(
                                [128, 512], dt2, tag="en", name=f"en{b}_{lh}_{ht}"
                            )
                            nc.scalar.activation(
                                en_t[:], pe_t[:], AF.Tanh,
                                bias=cb_sb[ht][:, b : b + 1],
                            )
                            ens.append(en_t)
                        if dve2 and b < BL - 2:
                            # z[p, l] = sum_ht v_ht[p] * en_ht[p, l]  (DVE).
                            # Intermediates accumulate in plain f32; only the
                            # final tile is written as dt2 for the ones-matmul.
                            z = None
                            for ht in range(4):
                                zn = stpool.tile(
                                    [128, 512], dt2 if ht == 3 else F32, tag="z",
                                    name=f"z{b}_{lh}_{ht}", bufs=8,
                                )
                                if z is None:
                                    nc.vector.tensor_scalar_mul(
                                        zn[:], ens[ht][:], vcol_sb[:, ht : ht + 1]
                                    )
                                else:
                                    nc.vector.scalar_tensor_tensor(
                                        zn[:], ens[ht][:],
                                        vcol_sb[:, ht : ht + 1], z[:],
                                        AluOpType.mult, AluOpType.add,
                                    )
                                z = zn
                            # scores[l] = sum_p z[p, l]: one K=128 ones-matmul
                            ps_s = ps3.tile(
                                [1, 512], F32, tag="pss", name=f"ps_s{b}_{lh}"
                            )
                            nc.tensor.matmul(
                                ps_s[:], lhsT=ones128[:], rhs=z[:],
                                start=True, stop=True,
                            )
                        else:
                            vc = vcolr_sb if dve2 else vcol_sb
                            pspool = ps3 if dve2 else ps2
                            ps_s = pspool.tile(
                                [1, 512], F32, tag="pss", name=f"ps_s{b}_{lh}"
                            )
                            for ht in range(4):
                                nc.tensor.matmul(
                                    ps_s[:],
                                    lhsT=vc[:, ht : ht + 1],
                                    rhs=ens[ht][:],
                                    start=(ht == 0),
                                    stop=(ht == 3),
                                )
                        # stage psum scores out and park them batch-major
                        st = stpool.tile([1, 512], F32, tag="st", name=f"st{b}_{lh}")
                        nc.vector.tensor_copy(st[:], ps_s[:])
                        nc.sync.dma_start(
                            scores_sb[b : b + 1, lh * 512 : (lh + 1) * 512], st[:]
                        )

                # ---- one softmax over all local batches ----
                mx = spool.tile([BL, 1], F32, tag="mx", name="mx")
                nc.vector.tensor_reduce(
                    mx[:], scores_sb[:], axis=AxisListType.X, op=AluOpType.max,
                    negate=True,
                )
                ex = spool.tile([BL, L], F32, tag="ex", name="ex")
                sm = spool.tile([BL, 1], F32, tag="sm", name="sm")
                nc.scalar.activation(
                    ex[:], scores_sb[:], AF.Exp, bias=mx[:, 0:1],
                    accum_out=sm[:],
                )
                rc = spool.tile([BL, 1], F32, tag="rc", name="rc")
                nc.vector.reciprocal(rc[:], sm[:])
                oo = spool.tile([BL, L], F32, tag="oo", name="oo")
                nc.vector.tensor_scalar_mul(oo[:], ex[:], rc[:, 0:1])
                nc.sync.dma_start(out[:, :], oo[:])

    nc.compile()
    return nc


_cached_nc = None


def _prep_in_maps(hidden, encoder_outputs, W, b, v, np1=np.float32, np2=np.float32):
    hidden = np.ascontiguousarray(hidden, dtype=np.float32)
    W = np.ascontiguousarray(W, dtype=np.float32)
    b = np.ascontiguousarray(b, dtype=np.float32)
    v = np.ascontiguousarray(v, dtype=np.float32)
    # (L, B, D) -> (B, D, L), zero-padded to DP on the d axis
    e = np.asarray(encoder_outputs, dtype=np.float32)
    encT = np.zeros((B, DP, L), dtype=np1)
    encT[:, :DE, :] = e.transpose(1, 2, 0).astype(np1)
    wet = np.zeros((DP, H), dtype=np1)
    wet[:DE] = W[:, H:].T.astype(np1)                   # We.T (padded)
    wht = np.ascontiguousarray(W[:, :H].T)              # (512, 512)
    bcol = np.ascontiguousarray(b.reshape(NHT, 128).T)  # (128, 4)
    vcol = np.ascontiguousarray(v.reshape(NHT, 128).T).astype(np2)  # (128, 4)
    ones = np.ones((128, 1), dtype=np1)
    in_maps = []
    for c in range(N_CORES):
        sl = slice(c * BL, (c + 1) * BL)
        in_maps.append(
            {
                "enc": encT[sl],
                "hid": np.ascontiguousarray(hidden[sl].T),
                "wet": wet,
                "wht": wht,
                "bcol": bcol,
                "vcol": vcol,
                "ones": ones,
                "vcolr": vcol.astype(np1),
            }
        )
    return in_maps


def kernel(hidden, encoder_outputs, W, b, v):
    global _cached_nc
    if _cached_nc is None:
        _cached_nc = build(reps=1)
    in_maps = _prep_in_maps(hidden, encoder_outputs, W, b, v)
    res = bass_utils.run_bass_kernel_spmd(
        _cached_nc, in_maps, core_ids=list(range(N_CORES))
    )
    outs = np.concatenate([res.results[c]["out"] for c in range(N_CORES)], axis=0)
    return outs[:, None, :].astype(np.float32)



# revision 3
# speedup vs baseline: 1.3489x; 1.3489x over previous
"""Trainium2 Bass kernel for nn_Attn (Bahdanau-style attention scores).

Computation (per batch b of B=128):
    energy = tanh(enc[b] @ We.T + (hidden @ Wh.T)[b] + bias)   # (L, H)
    scores = energy @ v                                        # (L,)
    out[b] = softmax(scores)                                   # (1, L)

Sharding: batch data-parallel over 8 NeuronCores (16 batches/core);
weights replicated. Per core the dominant matmul runs in the [h, l]
orientation (contract over d=576) so the (hidden@Wh.T + bias) term
fuses into the tanh as a per-partition activation bias.

Precision split on the d-contraction (the PE column count is the
bottleneck; fp8 DoubleRow streams 2 k-rows/cycle, bf16 streams 1):
  - d 0..255   : fp8 e4m3 via one DoubleRow matmul  (256 k-rows, 2x rate)
  - d 256..639 : bf16 via 3 plain k-tiles           (of which 576+ is pad)
Host-side the fp8/bf16 operands are pre-scaled by 16 (enc) and 256 (We)
to dodge e4m3's tiny subnormal range; the tanh activation un-scales via
its `scale` immediate. Simulated end-to-end rel err: 1.3e-2 (fp8-only
would be 2.4e-2, over the 2e-2 gate; bf16-only 2.5e-3 but 0 PE win).

The N(free)=1024 tiles (both l-halves in one matmul/activation, legal
for 8/16-bit moving operands; psum tiles span 2 banks) halve the
instruction count on PE/ACT/DVE versus N=512.

Stage-2 (scores = v . energy) runs on the DVE as a per-partition
multiply-accumulate chain in bf16 (2x DVE rate), collapsed to scores by
a single K=128 ones-matmul per l-half; the last 2 batches instead use
direct PE v-matmuls so the kernel tail isn't gated on the DVE chain.

Scores are assembled batch-major so softmax runs once over all local
batches at the end.
"""

import numpy as np
import ml_dtypes

import concourse.bacc as bacc
import concourse.mybir as mybir
import concourse.tile as tile
from concourse import bass_utils
from concourse.mybir import ActivationFunctionType as AF
from concourse.mybir import AluOpType, AxisListType

N_CORES = 8
B, L, H = 128, 1024, 512
ONEHOT = 64
DE = H + ONEHOT          # 576, true contraction dim of the big matmul
BL = B // N_CORES        # 16 batches per core
F32 = mybir.dt.float32
BF16 = mybir.dt.bfloat16
F8 = mybir.dt.float8e4

K8 = 256                 # fp8 DoubleRow k-rows (d 0..255)
NBT = 3                  # bf16 k-tiles (d 256..639; 576..639 zero pad)
S_E = 16.0               # host-side enc scale before quantization
S_W = 256.0              # host-side We scale before quantization
INV_S = 1.0 / (S_E * S_W)
NF = 1024                # matmul moving free dim (both l-halves at once)

DR = mybir.MatmulPerfMode.DoubleRow


def build(reps: int = 1, nwarm: int = 28):
    """Build + trace the per-core Bass program. Returns the compiled nc."""
    nc = bacc.Bacc(
        "TRN2", target_bir_lowering=False, debug=False, num_devices=N_CORES
    )
    enc8 = nc.dram_tensor("enc8", [BL, 128, 2, 1024], F8, kind="ExternalInput").ap()
    encb = nc.dram_tensor("encb", [BL, 128, NBT, 1024], BF16, kind="ExternalInput").ap()
    hid = nc.dram_tensor("hid", [H, BL], F32, kind="ExternalInput").ap()
    wet8 = nc.dram_tensor("wet8", [128, 2, 512], F8, kind="ExternalInput").ap()
    wbt = nc.dram_tensor("wbt", [128, NBT, 512], BF16, kind="ExternalInput").ap()
    wht = nc.dram_tensor("wht", [H, H], F32, kind="ExternalInput").ap()
    bcol = nc.dram_tensor("bcol", [128, 4], F32, kind="ExternalInput").ap()
    vcol = nc.dram_tensor("vcol", [128, 4], F32, kind="ExternalInput").ap()
    ones = nc.dram_tensor("ones", [128, 1], BF16, kind="ExternalInput").ap()
    vcolr = nc.dram_tensor("vcolr", [128, 4], BF16, kind="ExternalInput").ap()
    out = nc.dram_tensor("out", [BL, L], F32, kind="ExternalOutput").ap()

    with tile.TileContext(nc) as tc:
        with (
            tc.tile_pool(name="const", bufs=1) as cpool,
            tc.tile_pool(name="encp", bufs=4) as epool,
            tc.tile_pool(name="energy", bufs=8) as gpool,
            tc.tile_pool(name="cb", bufs=2) as cbpool,
            tc.tile_pool(name="soft", bufs=1) as spool,
            tc.tile_pool(name="stage", bufs=4) as stpool,
            tc.tile_pool(name="ps1", bufs=3, space="PSUM") as ps1,
            tc.tile_pool(name="ps3", bufs=2, space="PSUM") as ps3,
        ):
            # ---- replicated constants ----
            wet8_sb = cpool.tile([128, 2, 512], F8, tag="wet8", name="wet8_sb")
            nc.sync.dma_start(wet8_sb[:], wet8[:, :, :])
            wbt_sb = cpool.tile([128, NBT, 512], BF16, tag="wbt", name="wbt_sb")
            nc.sync.dma_start(wbt_sb[:], wbt[:, :, :])
            wht_sb = []
            for kt in range(4):
                t = cpool.tile([128, H], F32, tag=f"wht{kt}", name=f"wht{kt}")
                nc.sync.dma_start(t[:], wht[kt * 128 : (kt + 1) * 128, :])
                wht_sb.append(t)
            hid_sb = []
            for kt in range(4):
                t = cpool.tile([128, BL], F32, tag=f"hid{kt}", name=f"hid{kt}")
                nc.sync.dma_start(t[:], hid[kt * 128 : (kt + 1) * 128, :])
                hid_sb.append(t)
            bcol_sb = cpool.tile([128, 4], F32, tag="bcol", name="bcol_sb")
            nc.sync.dma_start(bcol_sb[:], bcol[:, :])
            vcol_sb = cpool.tile([128, 4], F32, tag="vcol", name="vcol_sb")
            nc.sync.dma_start(vcol_sb[:], vcol[:, :])
            ones128 = cpool.tile([128, 1], BF16, tag="ones128", name="ones128")
            nc.sync.dma_start(ones128[:], ones[:, :])
            vcolr_sb = cpool.tile([128, 4], BF16, tag="vcolr", name="vcolr_sb")
            nc.sync.dma_start(vcolr_sb[:], vcolr[:, :])

            for _rep in range(reps):
                # ---- PE warmup: junk DoubleRow matmuls as soon as wet8
                # lands, so the HAM clock-gate reaches 8/8 before real work
                # and stays there over the initial enc DMA window.
                warm = ps1.tile([128, NF], F32, tag="ps1", name="warm")
                for w in range(nwarm):
                    nc.tensor.matmul(
                        warm[:, 0:512],
                        lhsT=wet8_sb[:, :, 0:128],
                        rhs=wet8_sb[:, :, :],
                        start=(w == 0),
                        stop=(w == nwarm - 1),
                        perf_mode=DR,
                    )

                # ---- c[h, b] = (hidden @ Wh.T).T + bias, per-partition h ----
                cb_sb = []
                for ht in range(4):
                    pc = ps1.tile([128, NF], F32, tag="ps1", name=f"pc{ht}")
                    for kt in range(4):
                        nc.tensor.matmul(
                            pc[:, :BL],
                            lhsT=wht_sb[kt][:, ht * 128 : (ht + 1) * 128],
                            rhs=hid_sb[kt][:],
                            start=(kt == 0),
                            stop=(kt == 3),
                        )
                    cbt = cbpool.tile([128, BL], F32, tag=f"cb{ht}", name=f"cb{ht}")
                    nc.vector.tensor_scalar_add(
                        cbt[:], pc[:, :BL], bcol_sb[:, ht : ht + 1]
                    )
                    cb_sb.append(cbt)

                scores_sb = spool.tile([BL, L], F32, tag="scores", name="scores_sb")

                # ---- main loop over local batches ----
                for b in range(BL):
                    e8t = epool.tile([128, 2, 1024], F8, tag="e8", name=f"e8_{b}")
                    nc.sync.dma_start(e8t[:], enc8[b])
                    ebt = epool.tile([128, NBT, 1024], BF16, tag="eb", name=f"eb_{b}")
                    nc.sync.dma_start(ebt[:], encb[b])

                    ens = []
                    for ht in range(4):
                        pe_t = ps1.tile([128, NF], F32, tag="ps1", name=f"pe{b}_{ht}")
                        # Two interleaved 512-wide accumulation groups (ISA
                        # caps a matmul's output at 512 elements = 1 psum
                        # bank); lh pairs share each LDWEIGHTS.
                        for lh in range(2):
                            nc.tensor.matmul(
                                pe_t[:, lh * 512 : (lh + 1) * 512],
                                lhsT=wet8_sb[:, :, ht * 128 : (ht + 1) * 128],
                                rhs=e8t[:, :, lh * 512 : (lh + 1) * 512],
                                start=True,
                                stop=False,
                                perf_mode=DR,
                            )
                        for kt in range(NBT):
                            for lh in range(2):
                                nc.tensor.matmul(
                                    pe_t[:, lh * 512 : (lh + 1) * 512],
                                    lhsT=wbt_sb[:, kt, ht * 128 : (ht + 1) * 128],
                                    rhs=ebt[:, kt, lh * 512 : (lh + 1) * 512],
                                    start=False,
                                    stop=(kt == NBT - 1),
                                )
                        en_t = gpool.tile([128, NF], BF16, tag="en", name=f"en{b}_{ht}")
                        nc.scalar.activation(
                            en_t[:], pe_t[:], AF.Tanh,
                            bias=cb_sb[ht][:, b : b + 1], scale=INV_S,
                        )
                        ens.append(en_t)

                    if b < BL - 2:
                        # z[p, l] = sum_ht v_ht[p] * en_ht[p, l]  (DVE, bf16)
                        z = None
                        for ht in range(4):
                            zn = stpool.tile(
                                [128, NF], BF16, tag="z", name=f"z{b}_{ht}", bufs=8
                            )
                            if z is None:
                                nc.vector.tensor_scalar_mul(
                                    zn[:], ens[ht][:], vcol_sb[:, ht : ht + 1]
                                )
                            else:
                                nc.vector.scalar_tensor_tensor(
                                    zn[:], ens[ht][:],
                                    vcol_sb[:, ht : ht + 1], z[:],
                                    AluOpType.mult, AluOpType.add,
                                )
                            z = zn
                        for lh in range(2):
                            ps_s = ps3.tile([1, 512], F32, tag="pss",
                                            name=f"ps_s{b}_{lh}")
                            nc.tensor.matmul(
                                ps_s[:], lhsT=ones128[:],
                                rhs=z[:, lh * 512 : (lh + 1) * 512],
                                start=True, stop=True,
                            )
                            st = stpool.tile([1, 512], F32, tag="st",
                                             name=f"st{b}_{lh}")
                            nc.vector.tensor_copy(st[:], ps_s[:])
                            nc.sync.dma_start(
                                scores_sb[b : b + 1, lh * 512 : (lh + 1) * 512],
                                st[:],
                            )
                    else:
                        for lh in range(2):
                            ps_s = ps3.tile([1, 512], F32, tag="pss",
                                            name=f"ps_s{b}_{lh}")
                            for ht in range(4):
                                nc.tensor.matmul(
                                    ps_s[:],
                                    lhsT=vcolr_sb[:, ht : ht + 1],
                                    rhs=ens[ht][:, lh * 512 : (lh + 1) * 512],
                                    start=(ht == 0),
                                    stop=(ht == 3),
                                )
                            st = stpool.tile([1, 512], F32, tag="st",
                                             name=f"st{b}_{lh}")
                            nc.vector.tensor_copy(st[:], ps_s[:])
                            nc.sync.dma_start(
                                scores_sb[b : b + 1, lh * 512 : (lh + 1) * 512],
                                st[:],
                            )

                # ---- one softmax over all local batches ----
                mx = spool.tile([BL, 1], F32, tag="mx", name="mx")
                nc.vector.tensor_reduce(
                    mx[:], scores_sb[:], axis=AxisListType.X, op=AluOpType.max,
                    negate=True,
                )
                ex = spool.tile([BL, L], F32, tag="ex", name="ex")
                sm = spool.tile([BL, 1], F32, tag="sm", name="sm")
                nc.scalar.activation(
                    ex[:], scores_sb[:], AF.Exp, bias=mx[:, 0:1],
                    accum_out=sm[:],
                )
                rc = spool.tile([BL, 1], F32, tag="rc", name="rc")
                nc.vector.reciprocal(rc[:], sm[:])
                oo = spool.tile([BL, L], F32, tag="oo", name="oo")
                nc.vector.tensor_scalar_mul(oo[:], ex[:], rc[:, 0:1])
                nc.sync.dma_start(out[:, :], oo[:])

    nc.compile()
    return nc


_cached_nc = None

_F8NP = ml_dtypes.float8_e4m3
_BFNP = ml_dtypes.bfloat16


def _prep_in_maps(hidden, encoder_outputs, W, b, v):
    hidden = np.ascontiguousarray(hidden, dtype=np.float32)
    W = np.ascontiguousarray(W, dtype=np.float32)
    b = np.ascontiguousarray(b, dtype=np.float32)
    v = np.ascontiguousarray(v, dtype=np.float32)
    e = np.asarray(encoder_outputs, dtype=np.float32)
    encT = e.transpose(1, 2, 0)                         # (B, D, L) view
    # fp8 part: d 0..255, DoubleRow-packed [b, p, i, l] with d = i*128 + p
    q8 = np.clip(encT[:, :K8, :] * S_E, -240, 240).astype(_F8NP)
    enc8 = np.ascontiguousarray(
        q8.reshape(B, 2, 128, L).transpose(0, 2, 1, 3)  # (B, 128, 2, L)
    )
    # bf16 part: d 256..639 (576.. zero pad), [b, p, kt, l] with d = 256+kt*128+p
    qb = np.zeros((B, NBT * 128, L), dtype=_BFNP)
    qb[:, : DE - K8] = (encT[:, K8:DE, :] * S_E).astype(_BFNP)
    encb = np.ascontiguousarray(qb.reshape(B, NBT, 128, L).transpose(0, 2, 1, 3))

    WeT = W[:, H:].T                                    # (D, H)
    w8 = np.clip(WeT[:K8] * S_W, -240, 240).astype(_F8NP)
    wet8 = np.ascontiguousarray(w8.reshape(2, 128, H).transpose(1, 0, 2))
    wb = np.zeros((NBT * 128, H), dtype=_BFNP)
    wb[: DE - K8] = (WeT[K8:DE] * S_W).astype(_BFNP)
    wbt = np.ascontiguousarray(wb.reshape(NBT, 128, H).transpose(1, 0, 2))

    wht = np.ascontiguousarray(W[:, :H].T)              # (512, 512)
    bcol = np.ascontiguousarray(b.reshape(4, 128).T)    # (128, 4)
    vcol = np.ascontiguousarray(v.reshape(4, 128).T)    # (128, 4)
    ones = np.ones((128, 1), dtype=_BFNP)
    in_maps = []
    for c in range(N_CORES):
        sl = slice(c * BL, (c + 1) * BL)
        in_maps.append(
            {
                "enc8": enc8[sl],
                "encb": encb[sl],
                "hid": np.ascontiguousarray(hidden[sl].T),
                "wet8": wet8,
                "wbt": wbt,
                "wht": wht,
                "bcol": bcol,
                "vcol": vcol,
                "ones": ones,
                "vcolr": vcol.astype(_BFNP),
            }
        )
    return in_maps


def kernel(hidden, encoder_outputs, W, b, v):
    global _cached_nc
    if _cached_nc is None:
        _cached_nc = build(reps=1)
    in_maps = _prep_in_maps(hidden, encoder_outputs, W, b, v)
    res = bass_utils.run_bass_kernel_spmd(
        _cached_nc, in_maps, core_ids=list(range(N_CORES))
    )
    outs = np.concatenate([res.results[c]["out"] for c in range(N_CORES)], axis=0)
    return outs[:, None, :].astype(np.float32)


# revision 20
# speedup vs baseline: 1.3875x; 1.0286x over previous
"""Trainium2 Bass kernel for nn_Attn (Bahdanau-style attention scores).

Computation (per batch b of B=128):
    energy = tanh(enc[b] @ We.T + (hidden @ Wh.T)[b] + bias)   # (L, H)
    scores = energy @ v                                        # (L,)
    out[b] = softmax(scores)                                   # (1, L)

Sharding: batch data-parallel over 8 NeuronCores (16 batches/core);
weights replicated. Per core the dominant matmul runs in the [h, l]
orientation (contract over d=576) so the (hidden@Wh.T + bias) term
fuses into the tanh as a per-partition activation bias.

Precision split on the d-contraction (the PE column count is the
bottleneck; fp8 DoubleRow streams 2 k-rows/cycle, bf16 streams 1):
  - d 0..255   : fp8 e4m3 via one DoubleRow matmul  (256 k-rows, 2x rate)
  - d 256..639 : bf16 via 3 plain k-tiles           (of which 576+ is pad)
Host-side the fp8/bf16 operands are pre-scaled by 16 (enc) and 256 (We)
to dodge e4m3's tiny subnormal range; the tanh activation un-scales via
its `scale` immediate. Simulated end-to-end rel err: 1.3e-2 (fp8-only
would be 2.4e-2, over the 2e-2 gate; bf16-only 2.5e-3 but 0 PE win).

The N(free)=1024 tiles (both l-halves in one matmul/activation, legal
for 8/16-bit moving operands; psum tiles span 2 banks) halve the
instruction count on PE/ACT/DVE versus N=512.

Stage-2 (scores = v . energy) runs on the DVE as a per-partition
multiply-accumulate chain in bf16 (2x DVE rate), collapsed to scores by
a single K=128 ones-matmul per l-half; the last 2 batches instead use
direct PE v-matmuls so the kernel tail isn't gated on the DVE chain.

Scores are assembled batch-major so softmax runs once over all local
batches at the end.
"""

import numpy as np
import ml_dtypes

import concourse.bacc as bacc
import concourse.mybir as mybir
import concourse.tile as tile
from concourse import bass_utils
from concourse.mybir import ActivationFunctionType as AF
from concourse.mybir import AluOpType, AxisListType

N_CORES = 8
B, L, H = 128, 1024, 512
ONEHOT = 64
DE = H + ONEHOT          # 576, true contraction dim of the big matmul
BL = B // N_CORES        # 16 batches per core
F32 = mybir.dt.float32
BF16 = mybir.dt.bfloat16
F8 = mybir.dt.float8e4

K8 = 256                 # fp8 DoubleRow k-rows (d 0..255)
NBT = 3                  # bf16 k-tiles (d 256..639; 576..639 zero pad)
S_E = 16.0               # host-side enc scale before quantization
S_W = 256.0              # host-side We scale before quantization
INV_S = 1.0 / (S_E * S_W)
NF = 1024                # matmul moving free dim (both l-halves at once)

DR = mybir.MatmulPerfMode.DoubleRow


def build(reps: int = 1, nwarm: int = 40):
    """Build + trace the per-core Bass program. Returns the compiled nc."""
    nc = bacc.Bacc(
        "TRN2", target_bir_lowering=False, debug=False, num_devices=N_CORES
    )
    enc8 = nc.dram_tensor("enc8", [BL, 128, 2, 1024], F8, kind="ExternalInput").ap()
    encb = nc.dram_tensor("encb", [BL, 128, NBT, 1024], BF16, kind="ExternalInput").ap()
    hid = nc.dram_tensor("hid", [H, BL], BF16, kind="ExternalInput").ap()
    wet8 = nc.dram_tensor("wet8", [128, 2, 512], F8, kind="ExternalInput").ap()
    wbt = nc.dram_tensor("wbt", [128, NBT, 512], BF16, kind="ExternalInput").ap()
    wht = nc.dram_tensor("wht", [H, H], BF16, kind="ExternalInput").ap()
    bcol = nc.dram_tensor("bcol", [128, 4], F32, kind="ExternalInput").ap()
    vcol = nc.dram_tensor("vcol", [128, 4], F32, kind="ExternalInput").ap()
    # mask8[:, 7] = ones, else 0; sliced [7-r : 15-r] it puts the ones
    # column at row r of an [8, N] matmul output (batch-major psum scores).
    mask8 = nc.dram_tensor("mask8", [128, 15], BF16, kind="ExternalInput").ap()
    # vmask8[:, ht, 7] = v[ht*128+p], else 0 (direct PE v-contraction path)
    vmask8 = nc.dram_tensor("vmask8", [128, 4, 15], BF16, kind="ExternalInput").ap()
    out = nc.dram_tensor("out", [BL, L], F32, kind="ExternalOutput").ap()

    with tile.TileContext(nc) as tc:
        with (
            tc.tile_pool(name="const", bufs=1) as cpool,
            tc.tile_pool(name="encp", bufs=4) as epool,
            tc.tile_pool(name="energy", bufs=8) as gpool,
            tc.tile_pool(name="cb", bufs=2) as cbpool,
            tc.tile_pool(name="soft", bufs=1) as spool,
            tc.tile_pool(name="stage", bufs=4) as stpool,
            tc.tile_pool(name="ps1", bufs=2, space="PSUM") as ps1,
            tc.tile_pool(name="ps3", bufs=1, space="PSUM") as ps3,
        ):
            # ---- replicated constants ----
            wet8_sb = cpool.tile([128, 2, 512], F8, tag="wet8", name="wet8_sb")
            nc.sync.dma_start(wet8_sb[:], wet8[:, :, :])
            wbt_sb = cpool.tile([128, NBT, 512], BF16, tag="wbt", name="wbt_sb")
            nc.sync.dma_start(wbt_sb[:], wbt[:, :, :])
            wht_sb = []
            for kt in range(4):
                t = cpool.tile([128, H], BF16, tag=f"wht{kt}", name=f"wht{kt}")
                nc.sync.dma_start(t[:], wht[kt * 128 : (kt + 1) * 128, :])
                wht_sb.append(t)
            hid_sb = []
            for kt in range(4):
                t = cpool.tile([128, BL], BF16, tag=f"hid{kt}", name=f"hid{kt}")
                nc.sync.dma_start(t[:], hid[kt * 128 : (kt + 1) * 128, :])
                hid_sb.append(t)
            bcol_sb = cpool.tile([128, 4], F32, tag="bcol", name="bcol_sb")
            nc.sync.dma_start(bcol_sb[:], bcol[:, :])
            vcol_sb = cpool.tile([128, 4], F32, tag="vcol", name="vcol_sb")
            nc.sync.dma_start(vcol_sb[:], vcol[:, :])
            mask8_sb = cpool.tile([128, 15], BF16, tag="mask8", name="mask8_sb")
            nc.sync.dma_start(mask8_sb[:], mask8[:, :])
            vmask8_sb = cpool.tile([128, 4, 15], BF16, tag="vmask8", name="vmask8_sb")
            nc.sync.dma_start(vmask8_sb[:], vmask8[:, :, :])

            for _rep in range(reps):
                # ---- PE warmup: junk DoubleRow matmuls as soon as wet8
                # lands, so the HAM clock-gate reaches 8/8 before real work
                # and stays there over the initial enc DMA window.
                warm = ps1.tile([128, NF], F32, tag="ps1", name="warm")
                for w in range(nwarm):
                    nc.tensor.matmul(
                        warm[:, 0:512],
                        lhsT=wet8_sb[:, :, 0:128],
                        rhs=wet8_sb[:, :, :],
                        start=(w == 0),
                        stop=(w == nwarm - 1),
                        perf_mode=DR,
                    )

                # ---- c[h, b] = (hidden @ Wh.T).T + bias, per-partition h ----
                cb_sb = []
                for ht in range(4):
                    pc = ps1.tile([128, NF], F32, tag="ps1", name=f"pc{ht}")
                    for kt in range(4):
                        nc.tensor.matmul(
                            pc[:, :BL],
                            lhsT=wht_sb[kt][:, ht * 128 : (ht + 1) * 128],
                            rhs=hid_sb[kt][:],
                            start=(kt == 0),
                            stop=(kt == 3),
                        )
                    cbt = cbpool.tile([128, BL], F32, tag=f"cb{ht}", name=f"cb{ht}")
                    nc.vector.tensor_scalar_add(
                        cbt[:], pc[:, :BL], bcol_sb[:, ht : ht + 1]
                    )
                    cb_sb.append(cbt)

                # batch-major psum scores, one [8, L] tile per half of the
                # local batches; each batch's stage-2 matmul accumulates
                # into its row via the shifted mask8/vmask8 columns.
                ps_sc = [
                    ps3.tile([8, L], F32, tag=f"pss{g}", name=f"ps_sc{g}")
                    for g in range(2)
                ]

                # ---- main loop over local batches ----
                for b in range(BL):
                    e8t = epool.tile([128, 2, 1024], F8, tag="e8", name=f"e8_{b}")
                    nc.sync.dma_start(e8t[:], enc8[b])
                    ebt = epool.tile([128, NBT, 1024], BF16, tag="eb", name=f"eb_{b}")
                    nc.sync.dma_start(ebt[:], encb[b])

                    ens = []
                    for ht in range(4):
                        pe_t = ps1.tile([128, NF], F32, tag="ps1", name=f"pe{b}_{ht}")
                        # Two interleaved 512-wide accumulation groups (ISA
                        # caps a matmul's output at 512 elements = 1 psum
                        # bank); lh pairs share each LDWEIGHTS.
                        for lh in range(2):
                            nc.tensor.matmul(
                                pe_t[:, lh * 512 : (lh + 1) * 512],
                                lhsT=wet8_sb[:, :, ht * 128 : (ht + 1) * 128],
                                rhs=e8t[:, :, lh * 512 : (lh + 1) * 512],
                                start=True,
                                stop=False,
                                perf_mode=DR,
                            )
                        for kt in range(NBT):
                            for lh in range(2):
                                nc.tensor.matmul(
                                    pe_t[:, lh * 512 : (lh + 1) * 512],
                                    lhsT=wbt_sb[:, kt, ht * 128 : (ht + 1) * 128],
                                    rhs=ebt[:, kt, lh * 512 : (lh + 1) * 512],
                                    start=False,
                                    stop=(kt == NBT - 1),
                                )
                        en_t = gpool.tile([128, NF], BF16, tag="en", name=f"en{b}_{ht}")
                        nc.scalar.activation(
                            en_t[:], pe_t[:], AF.Tanh,
                            bias=cb_sb[ht][:, b : b + 1], scale=INV_S,
                        )
                        ens.append(en_t)

                    g, r = b // 8, b % 8
                    msl = slice(7 - r, 15 - r)
                    if b % 8 != 7:
                        # z[p, l] = sum_ht v_ht[p] * en_ht[p, l]  (DVE, bf16)
                        z = None
                        for ht in range(4):
                            zn = stpool.tile(
                                [128, NF], BF16, tag="z", name=f"z{b}_{ht}", bufs=8
                            )
                            if z is None:
                                nc.vector.tensor_scalar_mul(
                                    zn[:], ens[ht][:], vcol_sb[:, ht : ht + 1]
                                )
                            else:
                                nc.vector.scalar_tensor_tensor(
                                    zn[:], ens[ht][:],
                                    vcol_sb[:, ht : ht + 1], z[:],
                                    AluOpType.mult, AluOpType.add,
                                )
                            z = zn
                        for lh in range(2):
                            nc.tensor.matmul(
                                ps_sc[g][:, lh * 512 : (lh + 1) * 512],
                                lhsT=mask8_sb[:, msl],
                                rhs=z[:, lh * 512 : (lh + 1) * 512],
                                start=(r == 0),
                                stop=False,
                            )
                    else:
                        # last batch of each half: contract v directly on the
                        # PE so the half's softmax isn't gated on the DVE chain
                        for lh in range(2):
                            for ht in range(4):
                                nc.tensor.matmul(
                                    ps_sc[g][:, lh * 512 : (lh + 1) * 512],
                                    lhsT=vmask8_sb[:, ht, msl],
                                    rhs=ens[ht][:, lh * 512 : (lh + 1) * 512],
                                    start=False,
                                    stop=(ht == 3),
                                )

                    # ---- softmax per half as soon as its scores land, so
                    # only the last half's softmax sits in the tail ----
                    if r == 7:
                        sl = slice(g * 8, g * 8 + 8)
                        mx = spool.tile([8, 1], F32, tag=f"mx{g}", name=f"mx{g}")
                        nc.vector.tensor_reduce(
                            mx[:], ps_sc[g][:, :], axis=AxisListType.X,
                            op=AluOpType.max, negate=True,
                        )
                        ex = spool.tile([8, L], F32, tag=f"ex{g}", name=f"ex{g}")
                        sm = spool.tile([8, 1], F32, tag=f"sm{g}", name=f"sm{g}")
                        nc.scalar.activation(
                            ex[:], ps_sc[g][:, :], AF.Exp, bias=mx[:, 0:1],
                            accum_out=sm[:],
                        )
                        rc = spool.tile([8, 1], F32, tag=f"rc{g}", name=f"rc{g}")
                        nc.vector.reciprocal(rc[:], sm[:])
                        oo = spool.tile([8, L], F32, tag=f"oo{g}", name=f"oo{g}")
                        nc.vector.tensor_scalar_mul(
                            oo[:], ex[:], rc[:, 0:1]
                        )
                        nc.sync.dma_start(out[sl, :], oo[:])

    nc.compile()
    return nc


_cached_nc = None

_F8NP = ml_dtypes.float8_e4m3
_BFNP = ml_dtypes.bfloat16


def _prep_in_maps(hidden, encoder_outputs, W, b, v):
    hidden = np.ascontiguousarray(hidden, dtype=np.float32)
    W = np.ascontiguousarray(W, dtype=np.float32)
    b = np.ascontiguousarray(b, dtype=np.float32)
    v = np.ascontiguousarray(v, dtype=np.float32)
    e = np.asarray(encoder_outputs, dtype=np.float32)
    encT = e.transpose(1, 2, 0)                         # (B, D, L) view
    # fp8 part: d 0..255, DoubleRow-packed [b, p, i, l] with d = i*128 + p
    q8 = np.clip(encT[:, :K8, :] * S_E, -240, 240).astype(_F8NP)
    enc8 = np.ascontiguousarray(
        q8.reshape(B, 2, 128, L).transpose(0, 2, 1, 3)  # (B, 128, 2, L)
    )
    # bf16 part: d 256..639 (576.. zero pad), [b, p, kt, l] with d = 256+kt*128+p
    qb = np.zeros((B, NBT * 128, L), dtype=_BFNP)
    qb[:, : DE - K8] = (encT[:, K8:DE, :] * S_E).astype(_BFNP)
    encb = np.ascontiguousarray(qb.reshape(B, NBT, 128, L).transpose(0, 2, 1, 3))

    WeT = W[:, H:].T                                    # (D, H)
    w8 = np.clip(WeT[:K8] * S_W, -240, 240).astype(_F8NP)
    wet8 = np.ascontiguousarray(w8.reshape(2, 128, H).transpose(1, 0, 2))
    wb = np.zeros((NBT * 128, H), dtype=_BFNP)
    wb[: DE - K8] = (WeT[K8:DE] * S_W).astype(_BFNP)
    wbt = np.ascontiguousarray(wb.reshape(NBT, 128, H).transpose(1, 0, 2))

    wht = np.ascontiguousarray(W[:, :H].T).astype(_BFNP)  # (512, 512)
    bcol = np.ascontiguousarray(b.reshape(4, 128).T)      # (128, 4)
    vcol = np.ascontiguousarray(v.reshape(4, 128).T)
    mask8 = np.zeros((128, 15), dtype=_BFNP)
    mask8[:, 7] = 1
    vmask8 = np.zeros((128, 4, 15), dtype=_BFNP)
    vmask8[:, :, 7] = vcol.astype(_BFNP)
    in_maps = []
    for c in range(N_CORES):
        sl = slice(c * BL, (c + 1) * BL)
        in_maps.append(
            {
                "enc8": enc8[sl],
                "encb": encb[sl],
                "hid": np.ascontiguousarray(hidden[sl].T).astype(_BFNP),
                "wet8": wet8,
                "wbt": wbt,
                "wht": wht,
                "bcol": bcol,
                "vcol": vcol,
                "mask8": mask8,
                "vmask8": vmask8,
            }
        )
    return in_maps


def kernel(hidden, encoder_outputs, W, b, v):
    global _cached_nc
    if _cached_nc is None:
        _cached_nc = build(reps=1)
    in_maps = _prep_in_maps(hidden, encoder_outputs, W, b, v)
    res = bass_utils.run_bass_kernel_spmd(
        _cached_nc, in_maps, core_ids=list(range(N_CORES))
    )
    outs = np.concatenate([res.results[c]["out"] for c in range(N_CORES)], axis=0)
    return outs[:, None, :].astype(np.float32)
